# revision 2
# baseline (speedup 1.0000x reference)
"""LIF spiking-neuron scan kernel for Trainium2 (Bass/Tile), 8-core SPMD.

Reference semantics (per element, T=4 sequential steps):
    mem = 0
    for t in range(T):
        mem = mem + x[t]
        s[t] = (mem >= 1.0)          # spike, exact 0.0/1.0 fp32
        mem = mem * (mem < 1.0)      # hard reset on spike

Sharding: x is [T*B, C, H, W] = [256, 128, 32, 32] fp32. Reshaped to
[T=4, B=64, C*H*W]; B is data-parallel sharded 8 ways (8 batch rows per
core). Each core's shard is viewed as [T, 128, 8192] fp32 (4 MiB per
timestep plane). The T-scan is local per core; no communication.

Per-core engine plan (memory-bound target):
  - DMA: 16 MiB in + 16 MiB out @ ~358 GB/s  -> ~94 us floor
  - DVE: per step  u = mem + x_t (tensor_tensor add),
                   s = (u >= 1)  (tensor_scalar is_ge, 2x mode),
                   mem = (u < 1) * u  (fused scalar_tensor_tensor)
         -> ~68 us, stays under the DMA roofline.
All membrane math is fp32 and bit-exact vs the jax reference:
mem*1.0+x == mem+x; (mem-1>=0) == (mem>=1) (Sterbenz); (1-s)*mem ==
(mem<1)*mem for s in {0,1}.
"""

import numpy as np

import concourse.bacc as bacc
import concourse.mybir as mybir
import concourse.tile as tile
from concourse.bass_utils import run_bass_kernel_spmd

T = 4
B = 64
CHW = 128 * 32 * 32  # 131072
N_CORES = 8
B_SHARD = B // N_CORES           # 8
ELEMS = B_SHARD * CHW            # 1048576 elems per timestep per core
P = 128
F = ELEMS // P                   # 8192
F_TILE = 4096                    # 2 MiB DMA per (t, j) tile
N_J = F // F_TILE

_cache = {}


def _build_module(f_tile=F_TILE, x_bufs=3, s_bufs=4, repeats=1, loop_n=0,
                  out_dt="uint8", use_accum=False, add_engine="vector",
                  t_major=False, chunks=None, isge_engine="vector",
                  isge_pool_every=0):
    if chunks is None:
        chunks = [f_tile] * (F // f_tile)
    assert sum(chunks) == F, chunks
    n_j = len(chunks)
    col_starts = [sum(chunks[:i]) for i in range(n_j)]
    max_chunk = max(chunks)
    odt = getattr(mybir.dt, out_dt)
    nc = bacc.Bacc("TRN2", target_bir_lowering=False, debug=False)
    x = nc.dram_tensor("x", (T, P, F), mybir.dt.float32, kind="ExternalInput").ap()
    out = nc.dram_tensor("out", (T, P, F), odt, kind="ExternalOutput").ap()

    fp32 = mybir.dt.float32
    Alu = mybir.AluOpType

    with tile.TileContext(nc) as tc:
        with (
            tc.tile_pool(name="xp", bufs=x_bufs) as xpool,
            tc.tile_pool(name="sp", bufs=s_bufs) as spool,
            tc.tile_pool(name="mp", bufs=n_j) as mpool,
        ):

            def body():
                for _ in range(repeats):
                    mems = {}
                    order = (
                        [(t, j) for t in range(T) for j in range(n_j)]
                        if t_major
                        else [(t, j) for j in range(n_j) for t in range(T)]
                    )
                    for t, j in order:
                        w = chunks[j]
                        sl = slice(col_starts[j], col_starts[j] + w)
                        if t == 0:
                            mem = mpool.tile([P, w], fp32, tag="mem")
                            mems[j] = mem
                        mem = mems[j]
                        if True:
                            if use_accum:
                                # x_t folds into mem during the DMA itself
                                # (SWDGE CCE add); no SBUF x tile, no DVE add.
                                if t == 0:
                                    nc.sync.dma_start(out=mem[:], in_=x[t, :, sl])
                                else:
                                    nc.gpsimd.dma_start(
                                        out=mem[:], in_=x[t, :, sl],
                                        accum_op=Alu.add,
                                    )
                                u = mem
                            else:
                                xt = xpool.tile([P, w], fp32, tag="x")
                                nc.sync.dma_start(out=xt[:], in_=x[t, :, sl])
                                if t == 0:
                                    u = xt
                                else:
                                    if add_engine == "gpsimd" or (
                                        add_engine == "split" and j % 2 == 0
                                    ):
                                        adder = nc.gpsimd
                                    else:
                                        adder = nc.vector
                                    adder.tensor_add(mem[:], mem[:], xt[:])
                                    u = mem
                            s = spool.tile([P, w], odt, tag="s")
                            cell = t * n_j + j
                            if (isge_pool_every and t < 2
                                    and cell % isge_pool_every == 0):
                                isge_eng = nc.gpsimd
                            else:
                                isge_eng = getattr(nc, isge_engine)
                            isge_eng.tensor_scalar(s[:], u[:], 1.0, None, Alu.is_ge)
                            if t < T - 1:
                                nc.vector.scalar_tensor_tensor(
                                    mem[:], u[:], 1.0, u[:], Alu.is_lt, Alu.mult
                                )
                            nc.sync.dma_start(out=out[t, :, sl], in_=s[:])

            if loop_n:
                with tc.For_i(0, loop_n, 1):
                    body()
            else:
                body()
    nc.compile()
    return nc


def _get_module():
    if "nc" not in _cache:
        _cache["nc"] = _build_module()
    return _cache["nc"]


def _shard_inputs(x_np):
    # x_np: [T*B, C, H, W] fp32 -> per-core [T, P, F]
    xr = np.ascontiguousarray(x_np).reshape(T, B, CHW)
    shards = []
    for k in range(N_CORES):
        sh = np.ascontiguousarray(xr[:, k * B_SHARD : (k + 1) * B_SHARD]).reshape(
            T, P, F
        )
        shards.append(sh)
    return shards


def _unshard_outputs(outs):
    # outs: list of [T, P, F] -> [T*B, C, H, W]
    full = np.empty((T, B, CHW), dtype=np.float32)
    for k, o in enumerate(outs):
        full[:, k * B_SHARD : (k + 1) * B_SHARD] = o.reshape(T, B_SHARD, CHW)
    return full.reshape(T * B, 128, 32, 32)


def run_traced(x_np, trace_cores=None):
    """Test-only: run with NTFF tracing, return BassKernelResults."""
    x_np = np.asarray(x_np, dtype=np.float32)
    nc = _get_module()
    shards = _shard_inputs(x_np)
    in_maps = [{"x": sh} for sh in shards]
    return run_bass_kernel_spmd(
        nc, in_maps, list(range(N_CORES)), trace=True,
        trace_cores=trace_cores,
    )


def kernel(x, T=4, **_unused):
    x_np = np.asarray(x, dtype=np.float32)
    assert int(T) == 4, f"kernel hardcoded for T=4, got {T}"
    assert x_np.shape == (256, 128, 32, 32), x_np.shape

    nc = _get_module()
    shards = _shard_inputs(x_np)
    in_maps = [{"x": sh} for sh in shards]
    res = run_bass_kernel_spmd(nc, in_maps, list(range(N_CORES)))
    outs = [r["out"] for r in res.results]
    return _unshard_outputs(outs)



# revision 4
# speedup vs baseline: 1.0325x; 1.0325x over previous
"""LIF spiking-neuron scan kernel for Trainium2 (Bass/Tile), 8-core SPMD.

Reference semantics (per element, T=4 sequential steps):
    mem = 0
    for t in range(T):
        mem = mem + x[t]
        s[t] = (mem >= 1.0)          # spike, exact 0.0/1.0 fp32
        mem = mem * (mem < 1.0)      # hard reset on spike
All membrane math is fp32 and bit-exact vs the jax reference:
mem*1.0+x == mem+x; (mem-1>=0) == (mem>=1); (1-s)*mem == (mem<1)*mem.

Sharding: x is [T*B, C, H, W] = [256, 128, 32, 32] fp32. Reshaped to
[T=4, B=64, C*H*W]; B is data-parallel sharded 8 ways (8 batch rows per
core). Each core's shard is viewed as [T, 128, 8192] fp32 (4 MiB per
timestep plane). The T-scan is local per core; no communication.

Per-core engine plan (memory-bound; HBM ~358 GB/s/core):
  - DMA in : 16 MiB of x                  -> ~47 us floor (dominant)
  - DVE    : u = mem + x_t (tensor_tensor add)
             s = (u >= 1) as bf16 (tensor_scalar is_ge, 2x_2P mode)
             mem = (u < 1) * u (fused scalar_tensor_tensor)   ~34 us
  - PE     : packs spikes 8/byte: psum[g, f] accumulates over t
             sum_t 2^(4k+t) * s_t[2g+k, f]  (diag-ish weights, exact
             integer arithmetic in bf16 x bf16 -> fp32 PSUM)
  - ScalarE: PSUM fp32 -> SBUF uint8 copies (exact, values <= 255)
  - DMA out: [64, 8192] uint8 = 0.5 MiB packed spikes  -> ~1.5 us
Host side unpacks bits (np.unpackbits) - format change only, all spike
computation happens on device.
"""

import numpy as np
import ml_dtypes

import concourse.bacc as bacc
import concourse.mybir as mybir
import concourse.tile as tile
from concourse.bass_utils import run_bass_kernel_spmd

T = 4
B = 64
CHW = 128 * 32 * 32  # 131072
N_CORES = 8
B_SHARD = B // N_CORES           # 8
ELEMS = B_SHARD * CHW            # 1048576 elems per timestep per core
P = 128
F = ELEMS // P                   # 8192
F_TILE = 4096                    # 2 MiB input DMA per (t, j) tile
MM_N = 512                       # matmul moving free dim (one PSUM bank)
M_OUT = 64                       # packed output partitions (128 / 2)

_cache = {}


def _weight_np():
    """W_t[p, m] = 2^(4*(p%2) + t) if p//2 == m else 0, laid out as one
    [128, T*64] bf16 SBUF tile (w[:, 64*t : 64*t+64] is W_t)."""
    w = np.zeros((P, T * M_OUT), dtype=np.float32)
    for p in range(P):
        for t in range(T):
            w[p, t * M_OUT + p // 2] = float(2 ** (4 * (p % 2) + t))
    return w.astype(ml_dtypes.bfloat16)


def _build_module(f_tile=F_TILE, x_bufs=3, s_bufs=3, repeats=1, loop_n=0,
                  pack=True, out_dt="uint8", add_engine="vector",
                  isge_engine="vector", copy_engine="scalar", psum_bufs=8,
                  pk_bufs=2):
    n_j = F // f_tile
    odt = getattr(mybir.dt, out_dt)
    nc = bacc.Bacc("TRN2", target_bir_lowering=False, debug=False)
    x = nc.dram_tensor("x", (T, P, F), mybir.dt.float32, kind="ExternalInput").ap()
    if pack:
        w = nc.dram_tensor(
            "w", (P, T * M_OUT), mybir.dt.bfloat16, kind="ExternalInput"
        ).ap()
        out = nc.dram_tensor("out", (M_OUT, F), mybir.dt.uint8,
                             kind="ExternalOutput").ap()
    else:
        out = nc.dram_tensor("out", (T, P, F), odt, kind="ExternalOutput").ap()

    fp32 = mybir.dt.float32
    bf16 = mybir.dt.bfloat16
    Alu = mybir.AluOpType

    with tile.TileContext(nc) as tc:
        with (
            tc.tile_pool(name="xp", bufs=x_bufs) as xpool,
            tc.tile_pool(name="sp", bufs=s_bufs) as spool,
            tc.tile_pool(name="mp", bufs=n_j) as mpool,
            tc.tile_pool(name="wp", bufs=1) as wpool,
            tc.tile_pool(name="pk", bufs=pk_bufs) as pkpool,
            tc.tile_pool(name="ps", bufs=psum_bufs, space="PSUM") as pspool,
        ):
            if pack:
                wt = wpool.tile([P, T * M_OUT], bf16, tag="w")
                nc.sync.dma_start(out=wt[:], in_=w[:, :])

            def body():
                for _ in range(repeats):
                    for j in range(n_j):
                        sl = slice(j * f_tile, (j + 1) * f_tile)
                        mem = mpool.tile([P, f_tile], fp32, tag="mem")
                        psums = []
                        if pack:
                            for c in range(f_tile // MM_N):
                                ps = pspool.tile([M_OUT, MM_N], fp32, tag="ps",
                                                 name=f"ps_{j}_{c}")
                                psums.append(ps)
                        for t in range(T):
                            xt = xpool.tile([P, f_tile], fp32, tag="x")
                            nc.sync.dma_start(out=xt[:], in_=x[t, :, sl])
                            if t == 0:
                                u = xt
                            else:
                                adder = getattr(nc, add_engine)
                                adder.tensor_add(mem[:], mem[:], xt[:])
                                u = mem
                            s = spool.tile([P, f_tile], bf16 if pack else odt,
                                           tag="s")
                            isge = getattr(nc, isge_engine)
                            isge.tensor_scalar(s[:], u[:], 1.0, None, Alu.is_ge)
                            if t < T - 1:
                                nc.vector.scalar_tensor_tensor(
                                    mem[:], u[:], 1.0, u[:], Alu.is_lt, Alu.mult
                                )
                            if pack:
                                for c in range(f_tile // MM_N):
                                    nc.tensor.matmul(
                                        psums[c][:],
                                        wt[:, t * M_OUT:(t + 1) * M_OUT],
                                        s[:, c * MM_N:(c + 1) * MM_N],
                                        start=(t == 0),
                                        stop=(t == T - 1),
                                    )
                            else:
                                nc.sync.dma_start(out=out[t, :, sl], in_=s[:])
                        if pack:
                            pk = pkpool.tile([M_OUT, f_tile], mybir.dt.uint8,
                                             tag="pk")
                            cpeng = getattr(nc, copy_engine)
                            for c in range(f_tile // MM_N):
                                csl = slice(c * MM_N, (c + 1) * MM_N)
                                if copy_engine == "scalar":
                                    cpeng.copy(pk[:, csl], psums[c][:])
                                else:
                                    cpeng.tensor_copy(pk[:, csl], psums[c][:])
                            nc.sync.dma_start(out=out[:, sl], in_=pk[:])

            if loop_n:
                with tc.For_i(0, loop_n, 1):
                    body()
            else:
                body()
    nc.compile()
    return nc


def _get_module():
    if "nc" not in _cache:
        _cache["nc"] = _build_module()
    return _cache["nc"]


def _shard_inputs(x_np):
    # x_np: [T*B, C, H, W] fp32 -> per-core [T, P, F]
    xr = np.ascontiguousarray(x_np).reshape(T, B, CHW)
    shards = []
    for k in range(N_CORES):
        sh = np.ascontiguousarray(xr[:, k * B_SHARD : (k + 1) * B_SHARD]).reshape(
            T, P, F
        )
        shards.append(sh)
    return shards


def _unpack_bits(o):
    # o: [64, F] uint8; byte[g, f] bit (4k+t) = s_t[2g+k, f]
    bits = np.unpackbits(o[:, None, :], axis=1, bitorder="little")  # [64,8,F]
    s = bits.reshape(M_OUT, 2, T, F).transpose(2, 0, 1, 3).reshape(T, P, F)
    return s


def _unshard_outputs(outs, packed=True):
    full = np.empty((T, B, CHW), dtype=np.float32)
    for k, o in enumerate(outs):
        s = _unpack_bits(o) if packed else o
        full[:, k * B_SHARD : (k + 1) * B_SHARD] = s.reshape(T, B_SHARD, CHW)
    return full.reshape(T * B, 128, 32, 32)


def kernel(x, T=4, **_unused):
    x_np = np.asarray(x, dtype=np.float32)
    assert int(T) == 4, f"kernel hardcoded for T=4, got {T}"
    assert x_np.shape == (256, 128, 32, 32), x_np.shape

    nc = _get_module()
    shards = _shard_inputs(x_np)
    w_np = _weight_np()
    in_maps = [{"x": sh, "w": w_np} for sh in shards]
    res = run_bass_kernel_spmd(nc, in_maps, list(range(N_CORES)))
    outs = [r["out"] for r in res.results]
    return _unshard_outputs(outs)
